# revision 61
# baseline (speedup 1.0000x reference)
"""Trainium2 Bass kernel for causal multi-head attention with RoPE + register tokens.

Problem (nn_Attention_38293928411140):
  B=1, S=4096, HIDDEN=512, 8 heads x head_dim 64, causal SDPA, RoPE applied to
  positions >= num_registers (cos/sin indexed by position - num_registers), fp32.
  out = softmax(causal(QK^T/8)) V followed by a Wo projection.

Sharding: tensor-parallel over heads -- one head per NeuronCore (8 heads, 8 cores).

v2 per-core kernel (bf16 dataflow + fp8 DoubleRow scores), cost-model 110623 ns
vs the 172881 ns fp32r v1 (1.56x), measured rel err 8.9e-3 vs fp32 reference:
  - X^T built on-chip per 512-column chunk (bf16 PE transposes, 1.0 cyc/row),
    chunk DMA issued a supertile early, prep pipelined just-in-time 2 ahead
  - Q/K projections stacked with their rotate_half-premultiplied twins
    ([Wq; rot(Wq)] -> PSUM rows [q | qrot]); one DVE multiply against a
    stacked [cos; sin] table yields both RoPE product terms at once
  - scores via a 256-contraction identity:
      (q1+q2)(k1+k2) = [q1;q1;q2;q2] . [k1;k2;k1;k2]
    with both operands quantized fp8e4 (chunk-major [p,c,j,s] layout so
    range-based dependency tracking sees no false overlaps; the partition-
    shifted duplicate halves are written by tiny SBUF-to-SBUF DMAs) and fed
    to the PE in DoubleRow mode: 0.5 cycles/row and no rope adds anywhere
  - exp on ACT only; score groups ping-pong between a 3-bank and a 2-bank
    PSUM pool (3/2 kp-chunks per exp instruction, ~370 ns fixed cost each);
    diagonal supertiles packed exact-causal (512/384/128/256 at contiguous
    offsets) so no masked column is ever exp'd
  - causal raggedness handled by affine_selects on [128,128] blocks only
  - AV in [q, d] orientation (ptile as stationary operand): full 128x128
    array utilization; one PSUM accumulation group per supertile (PSUM
    zeroing is 2KB-region granular, so qb slices share one start/stop);
    softmax row-sums land one per partition so normalization is a
    per-partition reciprocal + tensor_scalar multiply
  - emission is software-pipelined: group g+1's score matmuls enter the PE
    queue before group g's exp-dependent AV matmuls and before any prep, and
    the per-supertile output transposes are deferred into the next supertile
  - emission is pipelined 3 score-groups deep, across supertile boundaries
  - normalized output transposed back to [d, s] (bf16) and exchanged by TWO
    AllToAlls split on s-columns, so the second exchange overlaps the first
    half's Wo matmuls; dummy PE transposes keep the p-state ramped through
    the exchange; Wo accumulates ci-outer in ss-pairs over the otall loads.
Host side only packs weights (per-head slices, rotate_half fold, bf16 casts,
identity, full-length stacked cos/sin with identity rotation for register
tokens) and concatenates the 8 output shards.

A post-scheduling pass hoists extra semaphore waits onto sequencer no-ops
because this walrus build rejects instructions with more than one sync wait.
"""
import math
import numpy as np

import concourse.bass as bass
import concourse.mybir as mybir
import concourse.tile as tile
from concourse.bass_utils import run_bass_kernel_spmd

F32 = mybir.dt.float32
BF16 = mybir.dt.bfloat16
FP8 = mybir.dt.float8e4

HIDDEN = 512
NHEADS = 8
HD = 64
NCORES = 8
SCALE = 1.0 / math.sqrt(HD)

_PROGRAM_CACHE = {}




def _split_matmul_waits(nc):
    """Walrus's CoreV3 codegen rejects instructions carrying more than one sync
    wait ('Too many sync wait commands'). Hoist all but one wait onto
    same-engine sequencer no-ops inserted right before the instruction."""
    import bass_rust
    for f in nc.m.functions:
        for blk in f.blocks:
            out = []
            for inst in blk.instructions:
                si = getattr(inst, "sync_info", None)
                eng = getattr(inst, "engine", None)
                if si is not None and eng is not None and len(si.on_wait) > 1:
                    waits = list(si.on_wait)
                    for k, w in enumerate(waits[:-1]):
                        nop = bass_rust.InstNoOp(
                            name=f"{inst.name}-hw{k}",
                            engine=eng,
                            text_hint="hoisted-wait",
                            sync_info=mybir.SyncInfo(on_wait=[w], on_update=[]),
                        )
                        out.append(nop)
                    inst.sync_info = mybir.SyncInfo(
                        on_wait=[waits[-1]], on_update=list(si.on_update))
                out.append(inst)
            blk.instructions = out


def build_program(S=4096, hoist=True, repeat=1, mock_cc=False, hw_loop=0,
                  warm_mms=160, warm_mms2=0):
    """Build the SPMD Bass program (same NEFF on all 8 cores)."""
    assert S % 512 == 0
    W = 512                      # q-supertile width == s-chunk width
    NSUP = S // W
    NST = S // 128
    SHARD = S // NCORES

    nc = bass.Bass("TRN2", target_bir_lowering=False, debug=False,
                   num_devices=NCORES)

    x = nc.dram_tensor("x", [S, HIDDEN], BF16, kind="ExternalInput").ap()
    wq_in = nc.dram_tensor("wq", [HIDDEN, 128], BF16, kind="ExternalInput").ap()
    wk_in = nc.dram_tensor("wk", [HIDDEN, 128], BF16, kind="ExternalInput").ap()
    wv_in = nc.dram_tensor("wv", [HIDDEN, HD], BF16, kind="ExternalInput").ap()
    wo_in = nc.dram_tensor("woT", [HIDDEN, HIDDEN], BF16, kind="ExternalInput").ap()
    cs_in = nc.dram_tensor("cs", [128, S], F32, kind="ExternalInput").ap()
    ident_in = nc.dram_tensor("ident", [128, 128], BF16, kind="ExternalInput").ap()
    out_shard = nc.dram_tensor("out_shard", [SHARD, HIDDEN], F32,
                               kind="ExternalOutput").ap()

    a2a_in = [nc.dram_tensor(f"a2a_in{h}", [NCORES, HD, SHARD // 2], BF16)
              for h in range(2)]
    a2a_out = [nc.dram_tensor(f"a2a_out{h}", [NCORES, HD, SHARD // 2], BF16)
               for h in range(2)]

    Exp = mybir.ActivationFunctionType.Exp
    DR = mybir.MatmulPerfMode.DoubleRow

    with tile.TileContext(nc) as tc:
      with tc.tile_pool(name="persist", bufs=1) as pp:
        ident = pp.tile([128, 128], BF16)
        cs_sb = pp.tile([128, S], F32, tag="cs")
        wq_sb = pp.tile([128, 4 * 128], BF16, tag="wq")
        wk_sb = pp.tile([128, 4 * 128], BF16, tag="wk")
        wv_sb = pp.tile([128, 4 * 64], BF16, tag="wv")
        wo_sb = pp.tile([128, 4 * 512], BF16, tag="wo")
        qt8 = pp.tile([128, 2 * S], FP8, tag="qt8")    # [p, j, s] j-major
        kt8 = pp.tile([128, 2 * S], FP8, tag="kt8")
        vext = pp.tile([128, NST * 65], BF16, tag="vext")  # V rows + ones col
        ones_sb = pp.tile([128, NST], BF16, tag="ones")

        # chunk-major layout [p, c, j, 512]: every scores read and every
        # prep write stays inside one chunk block, so range-based dependency
        # tracking never sees false cross-chunk overlaps.
        qt8v = qt8[:].rearrange("p (c j s) -> p c j s", j=2, s=512)
        kt8v = kt8[:].rearrange("p (c j s) -> p c j s", j=2, s=512)
        vextv = vext[:].rearrange("p (t c) -> p t c", c=65)

        # DMA ordering: x chunk 0 (issued by prep_a below) and the weights
        # needed first go in front; wo_sb is only needed in the epilogue and
        # is issued after the pipeline starts. Nothing issues on the scalar
        # (ACT) queue -- ACT runs exp exclusively.
        def load_w(dst, src, m, eng=None):
            (eng or nc.scalar).dma_start(
                dst.rearrange("p (j m) -> p j m", m=m),
                src.rearrange("(j p) m -> p j m", p=128))

        nc.gpsimd.dma_start(ident[:], ident_in)
        load_w(wq_sb[:], wq_in, 128)
        load_w(wk_sb[:], wk_in, 128)
        nc.scalar.dma_start(cs_sb[:, 0:1536], cs_in[:, 0:1536])
        load_w(wv_sb[:], wv_in, 64)
        nc.gpsimd.memset(ones_sb[:], 1.0)
        nc.gpsimd.tensor_copy(vextv[:, :, 64], ones_sb[:])

        def late_loads():
            nc.gpsimd.dma_start(cs_sb[:, 1536:S], cs_in[:, 1536:S])
            load_w(wo_sb[:], wo_in, 512, eng=nc.gpsimd)

        import contextlib
        loop_cm = tc.For_i(0, hw_loop, 1) if hw_loop else contextlib.nullcontext()
        with loop_cm:
          for _rep in range(repeat):
            with tc.tile_pool(name="xin", bufs=3) as pxin, \
                 tc.tile_pool(name="xtc", bufs=3) as pxtc, \
                 tc.tile_pool(name="rope", bufs=2) as prt, \
                 tc.tile_pool(name="pt", bufs=3) as ppt, \
                 tc.tile_pool(name="pscA", bufs=1, space="PSUM") as pscA, \
                 tc.tile_pool(name="pscB", bufs=1, space="PSUM") as pscB, \
                 tc.tile_pool(name="pprep", bufs=2, space="PSUM") as pprep, \
                 tc.tile_pool(name="psot", bufs=1, space="PSUM") as psot:
                # Scores ping-pong between two single-buffer PSUM pools of
                # asymmetric size (A: 3 banks / up to 3 kp-chunks or the whole
                # 1408-col exact-causal diagonal; B: 2 banks / up to 2). Strict
                # alternation = double buffering; bigger groups = fewer exp
                # instructions (ACT per-instruction overhead is ~370ns).
                psc_state = {"flip": 0}

                xtc_by_c = {}
                xg_by_c = {}

                def prep_dma(c, split=False):
                    """Issue the x chunk load early (a supertile ahead) so
                    the transposes never head-of-line block the PE queue.
                    split=True loads per-hj so startup transposes can begin
                    after the first quarter arrives."""
                    if c >= NSUP:
                        return
                    xg = pxin.tile([128, 4, HIDDEN], BF16, tag="xin", name="xg")
                    xsrc = x[c * 512:(c + 1) * 512, :].rearrange(
                        "(k p) h -> p k h", p=128)
                    if split:
                        for hj in range(4):
                            hs = slice(hj * 128, (hj + 1) * 128)
                            nc.sync.dma_start(xg[:, :, hs], xsrc[:, :, hs])
                    else:
                        nc.sync.dma_start(xg[:], xsrc)
                    xg_by_c[c] = xg

                def prep_a(c, hjs=(0, 1, 2, 3)):
                    """On-chip transpose of the x chunk (bf16)."""
                    if hjs[0] == 0:
                        xtc_by_c[c] = [pxtc.tile([128, 512], BF16,
                                                 tag=f"xt{hj}", name=f"xt{hj}")
                                       for hj in range(4)]
                    xg = xg_by_c[c]
                    xtc = xtc_by_c[c]
                    for hj in hjs:
                        ps = pprep.tile([128, 512], BF16, tag="prep",
                                        name="trp")
                        for k in range(4):
                            nc.tensor.transpose(
                                ps[:, k * 128:(k + 1) * 128],
                                xg[:, k, hj * 128:(hj + 1) * 128],
                                ident[:])
                        nc.vector.tensor_copy(xtc[hj][:], ps[:])
                    if hjs[-1] == 3:
                        xg_by_c.pop(c)

                def prep_q(c):
                    """Q projection (+rot twin) -> rope products -> fp8
                    score operand qt8 with DMA partition-dup."""
                    xtc = xtc_by_c[c]
                    cs = slice(c * 512, (c + 1) * 512)
                    pq = pprep.tile([128, 512], F32, tag="prep", name="pq")
                    for hj in range(4):
                        nc.tensor.matmul(
                            pq[:], lhsT=wq_sb[:, hj * 128:(hj + 1) * 128],
                            rhs=xtc[hj][:], start=(hj == 0), stop=(hj == 3))
                    # qt8 j=0 <- [q1 ; q1], j=1 <- [q2 ; q2]
                    nc.vector.tensor_mul(qt8v[0:64, c, 0, :], pq[0:64, :],
                                         cs_sb[0:64, cs])
                    nc.vector.tensor_mul(qt8v[64:128, c, 1, :],
                                         pq[64:128, :], cs_sb[64:128, cs])
                    nc.sync.dma_start(qt8v[64:128, c, 0, :],
                                      qt8v[0:64, c, 0, :])
                    (nc.scalar if c == 0 else nc.sync).dma_start(
                        qt8v[0:64, c, 1, :], qt8v[64:128, c, 1, :])

                def prep_k(c):
                    xtc = xtc_by_c[c]
                    cs = slice(c * 512, (c + 1) * 512)
                    pk = pprep.tile([128, 512], F32, tag="prep", name="pk")
                    for hj in range(4):
                        nc.tensor.matmul(
                            pk[:], lhsT=wk_sb[:, hj * 128:(hj + 1) * 128],
                            rhs=xtc[hj][:], start=(hj == 0), stop=(hj == 3))
                    # kt8 j=0 = j=1 = [k1 ; k2]
                    nc.vector.tensor_mul(kt8v[:, c, 0, :], pk[:], cs_sb[:, cs])
                    nc.sync.dma_start(kt8v[:, c, 1, :], kt8v[:, c, 0, :])

                def prep_b2(c):
                    """V projection directly in [s, d] rows -> vext."""
                    xtc = xtc_by_c.pop(c)
                    pv = pprep.tile([128, 256], F32, tag="prep", name="pv")
                    for sc in range(4):
                        for hj in range(4):
                            nc.tensor.matmul(
                                pv[:, sc * 64:(sc + 1) * 64],
                                lhsT=xtc[hj][:, sc * 128:(sc + 1) * 128],
                                rhs=wv_sb[:, hj * 64:(hj + 1) * 64],
                                start=(hj == 0), stop=(hj == 3))
                    nc.vector.tensor_copy(
                        vextv[:, 4 * c:4 * c + 4, 0:64],
                        pv[:].rearrange("p (t d) -> p t d", d=64))

                def sup_groups(sup):
                    """(pool, [(kp, width, qoff, psum_off)...]) groups under
                    strict A/B alternation. Diagonal is exact-causal: one
                    1408-col group on A (with a 128-col stale-PSUM gap at
                    [896:1024] that is exp'd but never read), or split
                    896+384 across B+A when parity lands on B."""
                    groups = []
                    rem = list(range(sup * 4))
                    flip = psc_state["flip"]
                    while rem:
                        cap = 3 if flip == 0 else 2
                        take, rem = rem[:cap], rem[cap:]
                        groups.append((flip, [(kp, 512, 0, 512 * j)
                                              for j, kp in enumerate(take)]))
                        flip ^= 1
                    d = sup * 4
                    diag = [(d, 512, 0, 0), (d + 1, 384, 128, 512),
                            (d + 3, 128, 384, 896), (d + 2, 256, 256, 1024)]
                    if sup == NSUP - 1:
                        # last supertile: always split the diagonal so the
                        # final exp (and the drain behind it) is small
                        groups.append((flip, diag[:2]))
                        groups.append((flip ^ 1, [(d + 3, 128, 384, 0),
                                                  (d + 2, 256, 256, 128)]))
                    elif flip == 0:
                        groups.append((0, diag))
                        flip = 1
                    else:
                        groups.append((1, diag[:2]))
                        groups.append((0, [(d + 3, 128, 384, 0),
                                           (d + 2, 256, 256, 128)]))
                        flip = 1
                    psc_state["flip"] = flip
                    return groups

                def emit_scores(sup, pool_id, grp):
                    if pool_id == 0:
                        sp = pscA.tile([128, 1536], F32, tag="scA", name="sp")
                    else:
                        sp = pscB.tile([128, 1024], F32, tag="scB", name="sp")
                    for (kp, w, qoff, o) in grp:
                        nc.tensor.matmul(
                            sp[:, o:o + w],
                            lhsT=kt8v[:, kp // 4, :,
                                      (kp % 4) * 128:(kp % 4 + 1) * 128],
                            rhs=qt8v[:, sup, :, qoff:qoff + w],
                            start=True, stop=True, perf_mode=DR)
                    return sp

                def emit_post(sup, otp, grp, sp, av_state):
                    gw = max(o + w for (_, w, _, o) in grp)
                    pt = ppt.tile([128, 1536], BF16, tag="pt", name="ptile")
                    nc.scalar.activation(pt[:, 0:gw], sp[:, 0:gw], Exp,
                                         scale=SCALE)
                    for (kp, w, qoff, o) in grp:
                        if kp >= sup * 4:
                            nc.gpsimd.affine_select(
                                out=pt[:, o:o + 128],
                                in_=pt[:, o:o + 128],
                                pattern=[[1, 128]],
                                compare_op=mybir.AluOpType.is_ge, fill=0.0,
                                base=0, channel_multiplier=-1)
                    for (kp, w, qoff, o) in grp:
                        for qb in range(4):
                            if qb * 128 < qoff:
                                continue
                            # One accumulation group per supertile: PSUM
                            # zeroing is 2KB-region granular, so only the
                            # sup's first AV matmul starts it and only the
                            # last one stops it; first writes to the other
                            # qb slices overwrite via the pending-zero bits.
                            av_state["n"] += 1
                            nc.tensor.matmul(
                                otp[:, qb * 65:qb * 65 + 65],
                                lhsT=pt[:, o + qb * 128 - qoff:
                                        o + qb * 128 - qoff + 128],
                                rhs=vextv[:, kp, :],
                                start=(av_state["n"] == 1),
                                stop=(av_state["n"] == av_state["total"]))

                def attn_tail_a(sup, otp):
                    otpv = otp.rearrange("p (q c) -> p q c", c=65)
                    rsb = prt.tile([128, 4], F32, tag="rsb", name="rsb")
                    nc.vector.reciprocal(rsb[:], otpv[:, :, 64])
                    onorm = prt.tile([128, 256], BF16, tag="onorm", name="onorm")
                    for qb in range(4):
                        nc.vector.tensor_scalar_mul(
                            onorm[:, qb * 64:(qb + 1) * 64],
                            otp[:, qb * 65:qb * 65 + 64],
                            rsb[:, qb:qb + 1])
                    return onorm

                def attn_tail_b(sup, onorm):
                    # halves interleaved so the first exchange-half DMA (and
                    # on the last supertile, the first AllToAll) fires before
                    # the second half is even transposed
                    otT = pprep.tile([64, 512], BF16, tag="prep", name="otT")
                    ots = prt.tile([64, 512], BF16, tag="ots", name="ots")
                    for h in range(2):
                        for qb in (2 * h, 2 * h + 1):
                            nc.tensor.transpose(
                                otT[:, qb * 128:(qb + 1) * 128],
                                onorm[:, qb * 64:(qb + 1) * 64],
                                ident[:])
                        nc.vector.tensor_copy(ots[:, h * 256:(h + 1) * 256],
                                              otT[:, h * 256:(h + 1) * 256])
                        nc.sync.dma_start(a2a_in[h].ap()[sup],
                                          ots[:, h * 256:(h + 1) * 256])

                prep_dma(0, split=True)
                prep_dma(1)
                prep_dma(2)
                prep_a(0)
                prep_q(0)
                prep_k(0)
                prep_b2(0)
                prep_a(1)
                prep_q(1)
                prep_k(1)
                prep_b2(1)
                late_loads()
                pending_tail = [None]
                all_groups = [sup_groups(s) for s in range(NSUP)]
                carry_sp = []
                for sup in range(NSUP):
                    groups = all_groups[sup]
                    ngroups = len(groups)
                    otp = psot.tile([128, 4 * 65], F32, tag="otp", name="otp")
                    total_av = sum(1 for _, grp in groups for (kp, w, qoff, o)
                                   in grp for qb in range(4)
                                   if qb * 128 >= qoff)
                    av_state = {"n": 0, "total": total_av}
                    nxt = sup + 2

                    def flush_tail():
                        if pending_tail[0] is not None:
                            attn_tail_b(*pending_tail[0])
                            pending_tail[0] = None

                    if nxt < NSUP:
                        pieces = [lambda: prep_dma(nxt + 1),
                                  lambda: prep_a(nxt, (0, 1)),
                                  lambda: prep_a(nxt, (2, 3)),
                                  lambda: prep_q(nxt),
                                  lambda: prep_k(nxt),
                                  lambda: prep_b2(nxt)]
                    else:
                        pieces = []
                    # software-pipelined emission, across supertiles too: the
                    # next group's scores go to the PE queue before this
                    # group's exp-dependent AV matmuls and before any prep,
                    # so the PE never head-of-line blocks the ACT exp stream.
                    sps = [None] * ngroups
                    ncarry = len(carry_sp)
                    for i, c in enumerate(carry_sp):
                        sps[i] = c
                    carry_sp = []
                    for gi in range(ncarry, ngroups):
                        if sps[gi] is None:
                            sps[gi] = emit_scores(sup, *groups[gi])
                        if gi > ncarry + 1:
                            break
                    for gi, grp in enumerate(groups):
                        nxt_g = gi + 3
                        if nxt_g < ngroups:
                            if sps[nxt_g] is None:
                                sps[nxt_g] = emit_scores(sup, *groups[nxt_g])
                        elif sup + 1 < NSUP:
                            ng = nxt_g - ngroups
                            if ng < 3 and ng < len(all_groups[sup + 1]) and \
                                    len(carry_sp) == ng:
                                carry_sp.append(emit_scores(
                                    sup + 1, *all_groups[sup + 1][ng]))
                        emit_post(sup, otp, grp[1], sps[gi], av_state)
                        if gi == 0:
                            flush_tail()
                        if pieces and (gi > 0 or ngroups == 1):
                            pieces.pop(0)()
                    for piece in pieces:
                        piece()
                    onorm = attn_tail_a(sup, otp)
                    pending_tail[0] = (sup, onorm)
                attn_tail_b(*pending_tail[0])
                pending_tail[0] = None

                # ---- exchange heads, output projection ----
                with tc.tile_pool(name="fin", bufs=1) as pf, \
                     tc.tile_pool(name="fout", bufs=2) as pfo:
                    # s-column-split head exchange: the second half's
                    # AllToAll overlaps the first half's Wo matmuls
                    for h in range(2):
                        if mock_cc:
                            nc.gpsimd.dma_start(a2a_out[h].ap(),
                                                a2a_in[h].ap())
                        else:
                            nc.gpsimd.collective_compute(
                                "AllToAll", mybir.AluOpType.bypass,
                                replica_groups=[list(range(NCORES))],
                                ins=[a2a_in[h].ap()], outs=[a2a_out[h].ap()])
                    # keep the PE p-state ramped through the exchange so the
                    # Wo matmuls run at full clock (dummy transposes with no
                    # data deps filling otherwise-idle PE time)
                    warm = pf.tile([128, 128], BF16, tag="warm")
                    wps = pprep.tile([128, 128], BF16, tag="prep", name="warm")
                    nc.vector.tensor_copy(warm[:], ident[:])
                    for _ in range(warm_mms):
                        nc.tensor.transpose(wps[:], warm[:], ident[:])
                    otall = pf.tile([128, 4 * SHARD], BF16, tag="otall")
                    otall_v = otall[:].rearrange("p (c s) -> p c s", c=4)
                    out_issuers = [nc.sync, nc.sync, nc.sync, nc.scalar]
                    for h in range(2):
                        a2a_pcs = a2a_out[h].ap().rearrange(
                            "e d s -> (e d) s").rearrange(
                            "(c p) s -> p c s", p=128)
                        (nc.sync if h == 0 else nc.scalar).dma_start(
                            otall_v[:, :, h * 256:(h + 1) * 256], a2a_pcs)
                    for h in range(2):
                        pos = [pscA.tile([128, 512], F32, tag="scA",
                                         name=f"po{2 * h}"),
                               pscB.tile([128, 512], F32, tag="scB",
                                         name=f"po{2 * h + 1}")]
                        for ci in range(4):
                            for d in range(2):
                                ss = 2 * h + d
                                nc.tensor.matmul(
                                    pos[d][:],
                                    lhsT=otall[:, ci * SHARD + ss * 128:
                                               ci * SHARD + (ss + 1) * 128],
                                    rhs=wo_sb[:, ci * 512:(ci + 1) * 512],
                                    start=(ci == 0), stop=(ci == 3))
                        if h == 0 and warm_mms2:
                            for _ in range(warm_mms2):
                                nc.tensor.transpose(wps[:], warm[:], ident[:])
                        for d in range(2):
                            ss = 2 * h + d
                            osb = pfo.tile([128, 512], F32, tag="osb",
                                           name="osb")
                            nc.vector.tensor_copy(osb[:], pos[d][:])
                            out_issuers[ss].dma_start(
                                out_shard[ss * 128:(ss + 1) * 128, :], osb[:])
    if hoist:
        _split_matmul_waits(nc)
    return nc


def get_program(S=4096):
    if S not in _PROGRAM_CACHE:
        _PROGRAM_CACHE[S] = build_program(S)
    return _PROGRAM_CACHE[S]


def make_in_maps(hidden_states, Wq, Wk, Wv, Wo, cos, sin, num_registers, S):
    """Host-side packing: per-head weight slices with the rotate_half fold
    stacked below ([W; rot(W)] transposed), bf16 casts, stacked [cos; sin]."""
    import ml_dtypes
    bf16 = ml_dtypes.bfloat16
    nr = int(num_registers)
    X = np.asarray(hidden_states, dtype=np.float32).reshape(S, HIDDEN)
    Wq = np.asarray(Wq, dtype=np.float32)
    Wk = np.asarray(Wk, dtype=np.float32)
    Wv = np.asarray(Wv, dtype=np.float32)
    Wo = np.asarray(Wo, dtype=np.float32)
    cos = np.asarray(cos, dtype=np.float32)
    sin = np.asarray(sin, dtype=np.float32)

    cos_full = np.ones((S, HD), np.float32)
    sin_full = np.zeros((S, HD), np.float32)
    if nr < S:
        cos_full[nr:] = cos[:S - nr]
        sin_full[nr:] = sin[:S - nr]
    cs = np.ascontiguousarray(
        np.concatenate([cos_full.T, sin_full.T], axis=0))      # [128, S]
    woT = np.ascontiguousarray(Wo.T).astype(bf16)
    Xb = np.ascontiguousarray(X).astype(bf16)
    identb = np.eye(128, dtype=np.float32).astype(bf16)

    def rot(Wh):
        return np.concatenate([-Wh[HD // 2:], Wh[:HD // 2]], axis=0)

    in_maps = []
    for c in range(NCORES):
        sl = slice(c * HD, (c + 1) * HD)
        Wq_h, Wk_h, Wv_h = Wq[sl], Wk[sl], Wv[sl]
        wq = np.ascontiguousarray(
            np.concatenate([Wq_h, rot(Wq_h)], axis=0).T).astype(bf16)
        wk = np.ascontiguousarray(
            np.concatenate([Wk_h, rot(Wk_h)], axis=0).T).astype(bf16)
        wv = np.ascontiguousarray(Wv_h.T).astype(bf16)
        in_maps.append({
            "x": Xb, "wq": wq, "wk": wk, "wv": wv,
            "woT": woT, "cs": cs, "ident": identb,
        })
    return in_maps


def kernel(hidden_states, Wq, Wk, Wv, Wo, cos, sin, num_registers):
    hidden_states = np.asarray(hidden_states)
    B, S, H = hidden_states.shape
    assert B == 1 and H == HIDDEN
    nc = get_program(S)
    in_maps = make_in_maps(hidden_states, Wq, Wk, Wv, Wo, cos, sin,
                           num_registers, S)
    res = run_bass_kernel_spmd(nc, in_maps, list(range(NCORES)))
    shards = [res.results[c]["out_shard"] for c in range(NCORES)]
    out = np.concatenate(shards, axis=0).reshape(1, S, HIDDEN)
    return out.astype(np.float32)


# revision 65
# speedup vs baseline: 1.0033x; 1.0033x over previous
"""Trainium2 Bass kernel for causal multi-head attention with RoPE + register tokens.

Problem (nn_Attention_38293928411140):
  B=1, S=4096, HIDDEN=512, 8 heads x head_dim 64, causal SDPA, RoPE applied to
  positions >= num_registers (cos/sin indexed by position - num_registers), fp32.
  out = softmax(causal(QK^T/8)) V followed by a Wo projection.

Sharding: tensor-parallel over heads -- one head per NeuronCore (8 heads, 8 cores).

v2 per-core kernel (bf16 dataflow + fp8 DoubleRow scores), cost-model 109943 ns
vs the 172881 ns fp32r v1 (1.57x), measured rel err 8.9e-3 vs fp32 reference:
  - X^T built on-chip per 512-column chunk (bf16 PE transposes, 1.0 cyc/row),
    chunk DMA issued a supertile early, prep pipelined just-in-time 2 ahead
  - Q/K projections stacked with their rotate_half-premultiplied twins
    ([Wq; rot(Wq)] -> PSUM rows [q | qrot]); one DVE multiply against a
    stacked [cos; sin] table yields both RoPE product terms at once
  - scores via a 256-contraction identity:
      (q1+q2)(k1+k2) = [q1;q1;q2;q2] . [k1;k2;k1;k2]
    with both operands quantized fp8e4 (chunk-major [p,c,j,s] layout so
    range-based dependency tracking sees no false overlaps; the partition-
    shifted duplicate halves are written by tiny SBUF-to-SBUF DMAs) and fed
    to the PE in DoubleRow mode: 0.5 cycles/row and no rope adds anywhere
  - exp on ACT only; score groups ping-pong between a 3-bank and a 2-bank
    PSUM pool (3/2 kp-chunks per exp instruction, ~370 ns fixed cost each);
    diagonal supertiles packed exact-causal (512/384/128/256 at contiguous
    offsets) so no masked column is ever exp'd
  - causal raggedness handled by affine_selects on [128,128] blocks only
  - AV in [q, d] orientation (ptile as stationary operand): full 128x128
    array utilization; one PSUM accumulation group per supertile (PSUM
    zeroing is 2KB-region granular, so qb slices share one start/stop);
    softmax row-sums land one per partition so normalization is a
    per-partition reciprocal + tensor_scalar multiply
  - emission is software-pipelined: group g+1's score matmuls enter the PE
    queue before group g's exp-dependent AV matmuls and before any prep, and
    the per-supertile output transposes are deferred into the next supertile
  - emission is pipelined 3 score-groups deep, across supertile boundaries
  - normalized output transposed back to [d, s] (bf16) and exchanged by TWO
    AllToAlls split on s-columns, so the second exchange overlaps the first
    half's Wo matmuls; dummy PE transposes keep the p-state ramped through
    the exchange; Wo accumulates ci-outer in ss-pairs over the otall loads.
Host side only packs weights (per-head slices, rotate_half fold, bf16 casts,
identity, full-length stacked cos/sin with identity rotation for register
tokens) and concatenates the 8 output shards.

A post-scheduling pass hoists extra semaphore waits onto sequencer no-ops
because this walrus build rejects instructions with more than one sync wait.
"""
import math
import numpy as np

import concourse.bass as bass
import concourse.mybir as mybir
import concourse.tile as tile
from concourse.bass_utils import run_bass_kernel_spmd

F32 = mybir.dt.float32
BF16 = mybir.dt.bfloat16
FP8 = mybir.dt.float8e4

HIDDEN = 512
NHEADS = 8
HD = 64
NCORES = 8
SCALE = 1.0 / math.sqrt(HD)

_PROGRAM_CACHE = {}




def _split_matmul_waits(nc):
    """Walrus's CoreV3 codegen rejects instructions carrying more than one sync
    wait ('Too many sync wait commands'). Hoist all but one wait onto
    same-engine sequencer no-ops inserted right before the instruction."""
    import bass_rust
    for f in nc.m.functions:
        for blk in f.blocks:
            out = []
            for inst in blk.instructions:
                si = getattr(inst, "sync_info", None)
                eng = getattr(inst, "engine", None)
                if si is not None and eng is not None and len(si.on_wait) > 1:
                    waits = list(si.on_wait)
                    for k, w in enumerate(waits[:-1]):
                        nop = bass_rust.InstNoOp(
                            name=f"{inst.name}-hw{k}",
                            engine=eng,
                            text_hint="hoisted-wait",
                            sync_info=mybir.SyncInfo(on_wait=[w], on_update=[]),
                        )
                        out.append(nop)
                    inst.sync_info = mybir.SyncInfo(
                        on_wait=[waits[-1]], on_update=list(si.on_update))
                out.append(inst)
            blk.instructions = out


def build_program(S=4096, hoist=True, repeat=1, mock_cc=False, hw_loop=0,
                  warm_mms=160, warm_mms2=0, warm0=0):
    """Build the SPMD Bass program (same NEFF on all 8 cores)."""
    assert S % 512 == 0
    W = 512                      # q-supertile width == s-chunk width
    NSUP = S // W
    NST = S // 128
    SHARD = S // NCORES

    nc = bass.Bass("TRN2", target_bir_lowering=False, debug=False,
                   num_devices=NCORES)

    x = nc.dram_tensor("x", [S, HIDDEN], BF16, kind="ExternalInput").ap()
    wq_in = nc.dram_tensor("wq", [HIDDEN, 128], BF16, kind="ExternalInput").ap()
    wk_in = nc.dram_tensor("wk", [HIDDEN, 128], BF16, kind="ExternalInput").ap()
    wv_in = nc.dram_tensor("wv", [HIDDEN, HD], BF16, kind="ExternalInput").ap()
    wo_in = nc.dram_tensor("woT", [HIDDEN, HIDDEN], BF16, kind="ExternalInput").ap()
    cs_in = nc.dram_tensor("cs", [128, S], F32, kind="ExternalInput").ap()
    ident_in = nc.dram_tensor("ident", [128, 128], BF16, kind="ExternalInput").ap()
    out_shard = nc.dram_tensor("out_shard", [SHARD, HIDDEN], F32,
                               kind="ExternalOutput").ap()

    a2a_in = [nc.dram_tensor(f"a2a_in{h}", [NCORES, HD, SHARD // 2], BF16)
              for h in range(2)]
    a2a_out = [nc.dram_tensor(f"a2a_out{h}", [NCORES, HD, SHARD // 2], BF16)
               for h in range(2)]

    Exp = mybir.ActivationFunctionType.Exp
    DR = mybir.MatmulPerfMode.DoubleRow

    with tile.TileContext(nc) as tc:
      with tc.tile_pool(name="persist", bufs=1) as pp:
        ident = pp.tile([128, 128], BF16)
        cs_sb = pp.tile([128, S], F32, tag="cs")
        wq_sb = pp.tile([128, 4 * 128], BF16, tag="wq")
        wk_sb = pp.tile([128, 4 * 128], BF16, tag="wk")
        wv_sb = pp.tile([128, 4 * 64], BF16, tag="wv")
        wo_sb = pp.tile([128, 4 * 512], BF16, tag="wo")
        qt8 = pp.tile([128, 2 * S], FP8, tag="qt8")    # [p, j, s] j-major
        kt8 = pp.tile([128, 2 * S], FP8, tag="kt8")
        vext = pp.tile([128, NST * 65], BF16, tag="vext")  # V rows + ones col
        ones_sb = pp.tile([128, NST], BF16, tag="ones")

        # chunk-major layout [p, c, j, 512]: every scores read and every
        # prep write stays inside one chunk block, so range-based dependency
        # tracking never sees false cross-chunk overlaps.
        qt8v = qt8[:].rearrange("p (c j s) -> p c j s", j=2, s=512)
        kt8v = kt8[:].rearrange("p (c j s) -> p c j s", j=2, s=512)
        vextv = vext[:].rearrange("p (t c) -> p t c", c=65)

        # DMA ordering: x chunk 0 (issued by prep_a below) and the weights
        # needed first go in front; wo_sb is only needed in the epilogue and
        # is issued after the pipeline starts. Nothing issues on the scalar
        # (ACT) queue -- ACT runs exp exclusively.
        def load_w(dst, src, m, eng=None):
            (eng or nc.scalar).dma_start(
                dst.rearrange("p (j m) -> p j m", m=m),
                src.rearrange("(j p) m -> p j m", p=128))

        nc.gpsimd.dma_start(ident[:], ident_in)
        load_w(wq_sb[:], wq_in, 128)
        load_w(wk_sb[:], wk_in, 128)
        nc.scalar.dma_start(cs_sb[:, 0:1536], cs_in[:, 0:1536])
        load_w(wv_sb[:], wv_in, 64)
        nc.gpsimd.memset(ones_sb[:], 1.0)
        nc.gpsimd.tensor_copy(vextv[:, :, 64], ones_sb[:])

        # preload the Exp activation table during startup dead time (the
        # ~1.3us table load otherwise lands on the first real softmax exp),
        # and ramp the PE p-state with dummy transposes so the first chunk's
        # prep matmuls run at full clock
        tpre = pp.tile([128, 4], F32, tag="tpre")
        nc.scalar.activation(tpre[:, 0:1], ident[:, 0:1], Exp, scale=1.0)

        def late_loads():
            nc.gpsimd.dma_start(cs_sb[:, 1536:S], cs_in[:, 1536:S])
            load_w(wo_sb[:], wo_in, 512, eng=nc.gpsimd)

        import contextlib
        loop_cm = tc.For_i(0, hw_loop, 1) if hw_loop else contextlib.nullcontext()
        with loop_cm:
          for _rep in range(repeat):
            with tc.tile_pool(name="xin", bufs=3) as pxin, \
                 tc.tile_pool(name="xtc", bufs=3) as pxtc, \
                 tc.tile_pool(name="rope", bufs=2) as prt, \
                 tc.tile_pool(name="pt", bufs=3) as ppt, \
                 tc.tile_pool(name="pscA", bufs=1, space="PSUM") as pscA, \
                 tc.tile_pool(name="pscB", bufs=1, space="PSUM") as pscB, \
                 tc.tile_pool(name="pprep", bufs=2, space="PSUM") as pprep, \
                 tc.tile_pool(name="psot", bufs=1, space="PSUM") as psot:
                # Scores ping-pong between two single-buffer PSUM pools of
                # asymmetric size (A: 3 banks / up to 3 kp-chunks or the whole
                # 1408-col exact-causal diagonal; B: 2 banks / up to 2). Strict
                # alternation = double buffering; bigger groups = fewer exp
                # instructions (ACT per-instruction overhead is ~370ns).
                psc_state = {"flip": 0}

                xtc_by_c = {}
                xg_by_c = {}

                def prep_dma(c, split=False):
                    """Issue the x chunk load early (a supertile ahead) so
                    the transposes never head-of-line block the PE queue.
                    split=True loads per-hj so startup transposes can begin
                    after the first quarter arrives."""
                    if c >= NSUP:
                        return
                    xg = pxin.tile([128, 4, HIDDEN], BF16, tag="xin", name="xg")
                    xsrc = x[c * 512:(c + 1) * 512, :].rearrange(
                        "(k p) h -> p k h", p=128)
                    if split:
                        for hj in range(4):
                            hs = slice(hj * 128, (hj + 1) * 128)
                            nc.sync.dma_start(xg[:, :, hs], xsrc[:, :, hs])
                    else:
                        nc.sync.dma_start(xg[:], xsrc)
                    xg_by_c[c] = xg

                def prep_a(c, hjs=(0, 1, 2, 3)):
                    """On-chip transpose of the x chunk (bf16)."""
                    if hjs[0] == 0:
                        xtc_by_c[c] = [pxtc.tile([128, 512], BF16,
                                                 tag=f"xt{hj}", name=f"xt{hj}")
                                       for hj in range(4)]
                    xg = xg_by_c[c]
                    xtc = xtc_by_c[c]
                    for hj in hjs:
                        ps = pprep.tile([128, 512], BF16, tag="prep",
                                        name="trp")
                        for k in range(4):
                            nc.tensor.transpose(
                                ps[:, k * 128:(k + 1) * 128],
                                xg[:, k, hj * 128:(hj + 1) * 128],
                                ident[:])
                        nc.vector.tensor_copy(xtc[hj][:], ps[:])
                    if hjs[-1] == 3:
                        xg_by_c.pop(c)

                def prep_q(c):
                    """Q projection (+rot twin) -> rope products -> fp8
                    score operand qt8 with DMA partition-dup."""
                    xtc = xtc_by_c[c]
                    cs = slice(c * 512, (c + 1) * 512)
                    pq = pprep.tile([128, 512], F32, tag="prep", name="pq")
                    for hj in range(4):
                        nc.tensor.matmul(
                            pq[:], lhsT=wq_sb[:, hj * 128:(hj + 1) * 128],
                            rhs=xtc[hj][:], start=(hj == 0), stop=(hj == 3))
                    # qt8 j=0 <- [q1 ; q1], j=1 <- [q2 ; q2]
                    nc.vector.tensor_mul(qt8v[0:64, c, 0, :], pq[0:64, :],
                                         cs_sb[0:64, cs])
                    nc.vector.tensor_mul(qt8v[64:128, c, 1, :],
                                         pq[64:128, :], cs_sb[64:128, cs])
                    nc.sync.dma_start(qt8v[64:128, c, 0, :],
                                      qt8v[0:64, c, 0, :])
                    (nc.scalar if c == 0 else nc.sync).dma_start(
                        qt8v[0:64, c, 1, :], qt8v[64:128, c, 1, :])

                def prep_k(c):
                    xtc = xtc_by_c[c]
                    cs = slice(c * 512, (c + 1) * 512)
                    pk = pprep.tile([128, 512], F32, tag="prep", name="pk")
                    for hj in range(4):
                        nc.tensor.matmul(
                            pk[:], lhsT=wk_sb[:, hj * 128:(hj + 1) * 128],
                            rhs=xtc[hj][:], start=(hj == 0), stop=(hj == 3))
                    # kt8 j=0 = j=1 = [k1 ; k2]
                    nc.vector.tensor_mul(kt8v[:, c, 0, :], pk[:], cs_sb[:, cs])
                    nc.sync.dma_start(kt8v[:, c, 1, :], kt8v[:, c, 0, :])

                def prep_b2(c):
                    """V projection directly in [s, d] rows -> vext."""
                    xtc = xtc_by_c.pop(c)
                    pv = pprep.tile([128, 256], F32, tag="prep", name="pv")
                    for sc in range(4):
                        for hj in range(4):
                            nc.tensor.matmul(
                                pv[:, sc * 64:(sc + 1) * 64],
                                lhsT=xtc[hj][:, sc * 128:(sc + 1) * 128],
                                rhs=wv_sb[:, hj * 64:(hj + 1) * 64],
                                start=(hj == 0), stop=(hj == 3))
                    nc.vector.tensor_copy(
                        vextv[:, 4 * c:4 * c + 4, 0:64],
                        pv[:].rearrange("p (t d) -> p t d", d=64))

                def sup_groups(sup):
                    """(pool, [(kp, width, qoff, psum_off)...]) groups under
                    strict A/B alternation. Diagonal is exact-causal: one
                    1408-col group on A (with a 128-col stale-PSUM gap at
                    [896:1024] that is exp'd but never read), or split
                    896+384 across B+A when parity lands on B."""
                    groups = []
                    rem = list(range(sup * 4))
                    flip = psc_state["flip"]
                    while rem:
                        cap = 3 if flip == 0 else 2
                        take, rem = rem[:cap], rem[cap:]
                        groups.append((flip, [(kp, 512, 0, 512 * j)
                                              for j, kp in enumerate(take)]))
                        flip ^= 1
                    d = sup * 4
                    diag = [(d, 512, 0, 0), (d + 1, 384, 128, 512),
                            (d + 3, 128, 384, 896), (d + 2, 256, 256, 1024)]
                    if sup == NSUP - 1:
                        # last supertile: always split the diagonal so the
                        # final exp (and the drain behind it) is small
                        groups.append((flip, diag[:2]))
                        groups.append((flip ^ 1, [(d + 3, 128, 384, 0),
                                                  (d + 2, 256, 256, 128)]))
                    elif flip == 0:
                        groups.append((0, diag))
                        flip = 1
                    else:
                        groups.append((1, diag[:2]))
                        groups.append((0, [(d + 3, 128, 384, 0),
                                           (d + 2, 256, 256, 128)]))
                        flip = 1
                    psc_state["flip"] = flip
                    return groups

                def emit_scores(sup, pool_id, grp):
                    if pool_id == 0:
                        sp = pscA.tile([128, 1536], F32, tag="scA", name="sp")
                    else:
                        sp = pscB.tile([128, 1024], F32, tag="scB", name="sp")
                    for (kp, w, qoff, o) in grp:
                        nc.tensor.matmul(
                            sp[:, o:o + w],
                            lhsT=kt8v[:, kp // 4, :,
                                      (kp % 4) * 128:(kp % 4 + 1) * 128],
                            rhs=qt8v[:, sup, :, qoff:qoff + w],
                            start=True, stop=True, perf_mode=DR)
                    return sp

                def emit_post(sup, otp, grp, sp, av_state):
                    gw = max(o + w for (_, w, _, o) in grp)
                    pt = ppt.tile([128, 1536], BF16, tag="pt", name="ptile")
                    nc.scalar.activation(pt[:, 0:gw], sp[:, 0:gw], Exp,
                                         scale=SCALE)
                    for (kp, w, qoff, o) in grp:
                        if kp >= sup * 4:
                            nc.gpsimd.affine_select(
                                out=pt[:, o:o + 128],
                                in_=pt[:, o:o + 128],
                                pattern=[[1, 128]],
                                compare_op=mybir.AluOpType.is_ge, fill=0.0,
                                base=0, channel_multiplier=-1)
                    for (kp, w, qoff, o) in grp:
                        for qb in range(4):
                            if qb * 128 < qoff:
                                continue
                            # One accumulation group per supertile: PSUM
                            # zeroing is 2KB-region granular, so only the
                            # sup's first AV matmul starts it and only the
                            # last one stops it; first writes to the other
                            # qb slices overwrite via the pending-zero bits.
                            av_state["n"] += 1
                            nc.tensor.matmul(
                                otp[:, qb * 65:qb * 65 + 65],
                                lhsT=pt[:, o + qb * 128 - qoff:
                                        o + qb * 128 - qoff + 128],
                                rhs=vextv[:, kp, :],
                                start=(av_state["n"] == 1),
                                stop=(av_state["n"] == av_state["total"]))

                def attn_tail_a(sup, otp):
                    otpv = otp.rearrange("p (q c) -> p q c", c=65)
                    rsb = prt.tile([128, 4], F32, tag="rsb", name="rsb")
                    nc.vector.reciprocal(rsb[:], otpv[:, :, 64])
                    onorm = prt.tile([128, 256], BF16, tag="onorm", name="onorm")
                    for qb in range(4):
                        nc.vector.tensor_scalar_mul(
                            onorm[:, qb * 64:(qb + 1) * 64],
                            otp[:, qb * 65:qb * 65 + 64],
                            rsb[:, qb:qb + 1])
                    return onorm

                def attn_tail_b(sup, onorm):
                    # halves interleaved so the first exchange-half DMA (and
                    # on the last supertile, the first AllToAll) fires before
                    # the second half is even transposed
                    otT = pprep.tile([64, 512], BF16, tag="prep", name="otT")
                    ots = prt.tile([64, 512], BF16, tag="ots", name="ots")
                    for h in range(2):
                        for qb in (2 * h, 2 * h + 1):
                            nc.tensor.transpose(
                                otT[:, qb * 128:(qb + 1) * 128],
                                onorm[:, qb * 64:(qb + 1) * 64],
                                ident[:])
                        nc.vector.tensor_copy(ots[:, h * 256:(h + 1) * 256],
                                              otT[:, h * 256:(h + 1) * 256])
                        nc.sync.dma_start(a2a_in[h].ap()[sup],
                                          ots[:, h * 256:(h + 1) * 256])

                if warm0:
                    wup = pprep.tile([128, 128], BF16, tag="prep", name="wup")
                    for _ in range(warm0):
                        nc.tensor.transpose(wup[:], ident[:], ident[:])
                prep_dma(0, split=True)
                prep_dma(1)
                prep_dma(2)
                prep_a(0)
                prep_q(0)
                prep_k(0)
                prep_b2(0)
                prep_a(1)
                prep_q(1)
                prep_k(1)
                prep_b2(1)
                late_loads()
                pending_tail = [None]
                all_groups = [sup_groups(s) for s in range(NSUP)]
                carry_sp = []
                for sup in range(NSUP):
                    groups = all_groups[sup]
                    ngroups = len(groups)
                    otp = psot.tile([128, 4 * 65], F32, tag="otp", name="otp")
                    total_av = sum(1 for _, grp in groups for (kp, w, qoff, o)
                                   in grp for qb in range(4)
                                   if qb * 128 >= qoff)
                    av_state = {"n": 0, "total": total_av}
                    nxt = sup + 2

                    def flush_tail():
                        if pending_tail[0] is not None:
                            attn_tail_b(*pending_tail[0])
                            pending_tail[0] = None

                    if nxt < NSUP:
                        pieces = [lambda: prep_dma(nxt + 1),
                                  lambda: prep_a(nxt, (0, 1)),
                                  lambda: prep_a(nxt, (2, 3)),
                                  lambda: prep_q(nxt),
                                  lambda: prep_k(nxt),
                                  lambda: prep_b2(nxt)]
                    else:
                        pieces = []
                    # software-pipelined emission, across supertiles too: the
                    # next group's scores go to the PE queue before this
                    # group's exp-dependent AV matmuls and before any prep,
                    # so the PE never head-of-line blocks the ACT exp stream.
                    sps = [None] * ngroups
                    ncarry = len(carry_sp)
                    for i, c in enumerate(carry_sp):
                        sps[i] = c
                    carry_sp = []
                    for gi in range(ncarry, ngroups):
                        if sps[gi] is None:
                            sps[gi] = emit_scores(sup, *groups[gi])
                        if gi > ncarry + 1:
                            break
                    for gi, grp in enumerate(groups):
                        nxt_g = gi + 3
                        if nxt_g < ngroups:
                            if sps[nxt_g] is None:
                                sps[nxt_g] = emit_scores(sup, *groups[nxt_g])
                        elif sup + 1 < NSUP:
                            ng = nxt_g - ngroups
                            if ng < 3 and ng < len(all_groups[sup + 1]) and \
                                    len(carry_sp) == ng:
                                carry_sp.append(emit_scores(
                                    sup + 1, *all_groups[sup + 1][ng]))
                        emit_post(sup, otp, grp[1], sps[gi], av_state)
                        if gi == 0:
                            flush_tail()
                        if pieces and (gi > 0 or ngroups == 1):
                            pieces.pop(0)()
                    for piece in pieces:
                        piece()
                    onorm = attn_tail_a(sup, otp)
                    pending_tail[0] = (sup, onorm)
                attn_tail_b(*pending_tail[0])
                pending_tail[0] = None

                # ---- exchange heads, output projection ----
                with tc.tile_pool(name="fin", bufs=1) as pf, \
                     tc.tile_pool(name="fout", bufs=2) as pfo:
                    # s-column-split head exchange: the second half's
                    # AllToAll overlaps the first half's Wo matmuls
                    for h in range(2):
                        if mock_cc:
                            nc.gpsimd.dma_start(a2a_out[h].ap(),
                                                a2a_in[h].ap())
                        else:
                            nc.gpsimd.collective_compute(
                                "AllToAll", mybir.AluOpType.bypass,
                                replica_groups=[list(range(NCORES))],
                                ins=[a2a_in[h].ap()], outs=[a2a_out[h].ap()])
                    # keep the PE p-state ramped through the exchange so the
                    # Wo matmuls run at full clock (dummy transposes with no
                    # data deps filling otherwise-idle PE time)
                    warm = pf.tile([128, 128], BF16, tag="warm")
                    wps = pprep.tile([128, 128], BF16, tag="prep", name="warm")
                    nc.vector.tensor_copy(warm[:], ident[:])
                    for _ in range(warm_mms):
                        nc.tensor.transpose(wps[:], warm[:], ident[:])
                    otall = pf.tile([128, 4 * SHARD], BF16, tag="otall")
                    otall_v = otall[:].rearrange("p (c s) -> p c s", c=4)
                    out_issuers = [nc.sync, nc.sync, nc.sync, nc.scalar]
                    for h in range(2):
                        a2a_pcs = a2a_out[h].ap().rearrange(
                            "e d s -> (e d) s").rearrange(
                            "(c p) s -> p c s", p=128)
                        (nc.sync if h == 0 else nc.scalar).dma_start(
                            otall_v[:, :, h * 256:(h + 1) * 256], a2a_pcs)
                    for h in range(2):
                        pos = [pscA.tile([128, 512], F32, tag="scA",
                                         name=f"po{2 * h}"),
                               pscB.tile([128, 512], F32, tag="scB",
                                         name=f"po{2 * h + 1}")]
                        for ci in range(4):
                            for d in range(2):
                                ss = 2 * h + d
                                nc.tensor.matmul(
                                    pos[d][:],
                                    lhsT=otall[:, ci * SHARD + ss * 128:
                                               ci * SHARD + (ss + 1) * 128],
                                    rhs=wo_sb[:, ci * 512:(ci + 1) * 512],
                                    start=(ci == 0), stop=(ci == 3))
                        if h == 0 and warm_mms2:
                            for _ in range(warm_mms2):
                                nc.tensor.transpose(wps[:], warm[:], ident[:])
                        for d in range(2):
                            ss = 2 * h + d
                            osb = pfo.tile([128, 512], F32, tag="osb",
                                           name="osb")
                            nc.vector.tensor_copy(osb[:], pos[d][:])
                            out_issuers[ss].dma_start(
                                out_shard[ss * 128:(ss + 1) * 128, :], osb[:])
    if hoist:
        _split_matmul_waits(nc)
    return nc


def get_program(S=4096):
    if S not in _PROGRAM_CACHE:
        _PROGRAM_CACHE[S] = build_program(S)
    return _PROGRAM_CACHE[S]


def make_in_maps(hidden_states, Wq, Wk, Wv, Wo, cos, sin, num_registers, S):
    """Host-side packing: per-head weight slices with the rotate_half fold
    stacked below ([W; rot(W)] transposed), bf16 casts, stacked [cos; sin]."""
    import ml_dtypes
    bf16 = ml_dtypes.bfloat16
    nr = int(num_registers)
    X = np.asarray(hidden_states, dtype=np.float32).reshape(S, HIDDEN)
    Wq = np.asarray(Wq, dtype=np.float32)
    Wk = np.asarray(Wk, dtype=np.float32)
    Wv = np.asarray(Wv, dtype=np.float32)
    Wo = np.asarray(Wo, dtype=np.float32)
    cos = np.asarray(cos, dtype=np.float32)
    sin = np.asarray(sin, dtype=np.float32)

    cos_full = np.ones((S, HD), np.float32)
    sin_full = np.zeros((S, HD), np.float32)
    if nr < S:
        cos_full[nr:] = cos[:S - nr]
        sin_full[nr:] = sin[:S - nr]
    cs = np.ascontiguousarray(
        np.concatenate([cos_full.T, sin_full.T], axis=0))      # [128, S]
    woT = np.ascontiguousarray(Wo.T).astype(bf16)
    Xb = np.ascontiguousarray(X).astype(bf16)
    identb = np.eye(128, dtype=np.float32).astype(bf16)

    def rot(Wh):
        return np.concatenate([-Wh[HD // 2:], Wh[:HD // 2]], axis=0)

    in_maps = []
    for c in range(NCORES):
        sl = slice(c * HD, (c + 1) * HD)
        Wq_h, Wk_h, Wv_h = Wq[sl], Wk[sl], Wv[sl]
        wq = np.ascontiguousarray(
            np.concatenate([Wq_h, rot(Wq_h)], axis=0).T).astype(bf16)
        wk = np.ascontiguousarray(
            np.concatenate([Wk_h, rot(Wk_h)], axis=0).T).astype(bf16)
        wv = np.ascontiguousarray(Wv_h.T).astype(bf16)
        in_maps.append({
            "x": Xb, "wq": wq, "wk": wk, "wv": wv,
            "woT": woT, "cs": cs, "ident": identb,
        })
    return in_maps


def kernel(hidden_states, Wq, Wk, Wv, Wo, cos, sin, num_registers):
    hidden_states = np.asarray(hidden_states)
    B, S, H = hidden_states.shape
    assert B == 1 and H == HIDDEN
    nc = get_program(S)
    in_maps = make_in_maps(hidden_states, Wq, Wk, Wv, Wo, cos, sin,
                           num_registers, S)
    res = run_bass_kernel_spmd(nc, in_maps, list(range(NCORES)))
    shards = [res.results[c]["out_shard"] for c in range(NCORES)]
    out = np.concatenate(shards, axis=0).reshape(1, S, HIDDEN)
    return out.astype(np.float32)
